# revision 29
# baseline (speedup 1.0000x reference)
"""Trainium2 Bass kernel for nn_MessageLayer (GNN message passing), 8 NeuronCores.

Reference computation:
    edge_mat = (edge_features @ W + b).reshape(E, 64, 16)
    messages = einsum('emh,eh->em', edge_mat, hidden[edge_sources])
    out      = segment_sum(messages, edge_targets, num_segments=10000)

Algebraic restructure (cuts FLOPs 32x): since aggregation is linear,
    out[n, m] = sum_{f,h} W[f, m*16+h] * C[n, f, h],
    C[n, f, h] = sum_{e: tgt(e)=n} ef[e, f] * hidden[src(e), h]

Structure (v2.3): per-target segments ("positions", split at 64) are packed
into full-array K=128 matmuls in two species:
  - BIG (33..64 edges): 2 row-slots of 64 x 4 ef column-classes
    = 8 positions/matmul, moving [128, 128]
  - SMALL (<=32 edges): 4 row-slots of 32 x 4 classes
    = 16 positions/matmul, moving [128, 256]
Stationary [128, 128]: row r of slot j holds the 4 class-edges' features at
column groups 32g..32g+32 (dense).  Moving: slot j's rows carry the 4
source-hidden vectors at cols 64j+16g+h, zeros elsewhere (slot separation;
zeros memset on-device, data DMA'd compactly per slot-band).
PSUM out: valid C-blocks at (32g+f, stripe 16s+h) with s%4 == g uniformly
across both species, garbage elsewhere.  Each bank (4 big or 2 small
matmuls) drains as two half-width [128, 256] f32->bf16 copies (DVE + ACT in
parallel) into the spread c_spread.
W-stage: 4 concurrent 32-row-strip matmul chains (one per class g), each
reading its valid columns via stride-64:  c_spread[32g:32g+32, (16g+h)::64]
-> [32, U], against a 4x-replicated W stationary [32f@32g, 64m] (both
m-halves at once, 16 accumulating h-phases into po_g [64, U] PSUM).

Sharding: node-ownership (scatter-reduce by target): core c owns nodes
[1250c, 1250c+1250) and receives exactly the edges targeting them, so no
collective is needed; host assembles per-position rows into final output.
All tensors bf16 on the wire/SBUF (f32 PSUM accumulate): rel-err ~3.5e-3
vs the 2e-2 gate.
"""
import numpy as np
from contextlib import ExitStack

N_NODES = 10000
N_EDGES = 320000
HID = 16
MSG = 64
EFD = 32
NCORES = 8
NPC = N_NODES // NCORES          # 1250 nodes owned per core
CPBUFS = 4                       # PSUM tiles for C banks (4 + 4 po = 8)

_CACHE = {}


def _bf16():
    import ml_dtypes
    return ml_dtypes.bfloat16


def _build_layout(edge_targets):
    """Per-core position lists (node, edge-ids, len<=64, sorted desc; all
    len>32 "big" positions precede the "small" ones) plus the SPMD-uniform
    grid: T_big 8-position matmuls then T_small 16-position matmuls."""
    segs_per_core, nbig_per_core = [], []
    for c in range(NCORES):
        lo = c * NPC
        mask = (edge_targets >= lo) & (edge_targets < lo + NPC)
        eids = np.nonzero(mask)[0]
        tgt = edge_targets[eids]
        order = np.argsort(tgt, kind="stable")
        eids = eids[order]
        tgt = tgt[order]
        segs = []
        uniq, starts = np.unique(tgt, return_index=True)
        bounds = list(starts) + [len(tgt)]
        for i, n in enumerate(uniq):
            s, e = bounds[i], bounds[i + 1]
            while e - s > 64:
                segs.append((int(n), eids[s:s + 64]))
                s += 64
            segs.append((int(n), eids[s:e]))
        segs.sort(key=lambda t: -len(t[1]))
        segs_per_core.append(segs)
        nbig_per_core.append(sum(1 for _, e in segs if len(e) > 32))

    T_big = -(-max(nbig_per_core) // 8)
    T_big = ((T_big + 3) // 4) * 4            # whole banks of 4 matmuls
    nsmall = max(len(s) - b for s, b in zip(segs_per_core, nbig_per_core))
    T_small = -(-nsmall // 16)
    T_small = ((T_small + 1) // 2) * 2        # whole banks of 2 matmuls
    U = 2 * T_big + 4 * T_small               # total position quads
    assert U <= 512, f"U={U} exceeds one PSUM bank"
    return segs_per_core, nbig_per_core, T_big, T_small, U


def _build_w2(W):
    # w2[32g+f, 64h+m] = W[f, m*16+h], replicated across the 4 class groups
    Wr = W.reshape(EFD, MSG, HID).transpose(0, 2, 1)   # [f, h, m]
    blk = np.ascontiguousarray(Wr.reshape(EFD, HID * MSG))
    return np.tile(blk, (4, 1)).astype(np.float32)     # [128, 1024]


def _pack_core(segs, nbig, T_big, T_small, w2, edge_features, edge_sources,
               hidden):
    """DRAM image per core, bf16:
      [128, T*128 st | T_big*64 mv-big | T_small*64 mv-small | 1024 w2]
    BIG position p<8*T_big (t=p//8, j=(p//4)%2, g=p%4):
      st[64j+r, t*128+32g+f];  mv-big band j at partitions 64j: [64j+r, t*64+16g+h]
    SMALL position q (t=q//16, j=(q//4)%4, g=q%4):
      st[32j+r, (T_big+t)*128+32g+f];  mv-small band j at partitions 32j."""
    T = T_big + T_small
    St = np.zeros((128, T * 128), dtype=np.float32)
    # moving data ships window-major with DRAM zeros (one rect per chunk)
    MvB = np.zeros((128, T_big * 128), dtype=np.float32)
    MvS = np.zeros((128, T_small * 256), dtype=np.float32)
    for i in range(len(segs)):
        _, eids = segs[i]
        k = len(eids)
        if i < nbig:
            t, j, g = i // 8, (i // 4) % 2, i % 4
            r0 = 64 * j
            St[r0:r0 + k, t * 128 + 32 * g:t * 128 + 32 * g + EFD] = \
                edge_features[eids]
            MvB[r0:r0 + k, t * 128 + 64 * j + 16 * g:
                t * 128 + 64 * j + 16 * g + HID] = hidden[edge_sources[eids]]
        else:
            q = i - nbig
            t, j, g = q // 16, (q // 4) % 4, q % 4
            r0 = 32 * j
            St[r0:r0 + k, (T_big + t) * 128 + 32 * g:
               (T_big + t) * 128 + 32 * g + EFD] = edge_features[eids]
            MvS[r0:r0 + k, t * 256 + 64 * j + 16 * g:
               t * 256 + 64 * j + 16 * g + HID] = hidden[edge_sources[eids]]
    D = np.concatenate([St, MvB, MvS, w2], axis=1)
    return np.ascontiguousarray(D.astype(_bf16()))


def _chunks(T, n, align):
    bs = [((round(k * T / n)) // align) * align for k in range(n)] + [T]
    bs[1] = max(bs[1], align) if T >= align else bs[1]
    return [(bs[k], bs[k + 1]) for k in range(n) if bs[k + 1] > bs[k]]


def _build_program(T_big, T_small, U):
    import concourse.tile as tile
    from concourse import bacc, mybir

    f32 = mybir.dt.float32
    bf16 = mybir.dt.bfloat16
    T = T_big + T_small
    ST_W = T * 128
    B_big = T_big // 4
    B = B_big + T_small // 2             # total PSUM bank-fills
    MVB_SB = T_big * 128                 # big region width in mv_sb

    nc = bacc.Bacc("TRN2", target_bir_lowering=False, debug=False,
                   num_devices=NCORES)
    data_dram = nc.dram_tensor(
        "data", [128, ST_W + T_big * 128 + T_small * 256 + 1024], bf16,
        kind="ExternalInput").ap()
    out_dram = nc.dram_tensor("out", [128, 2 * U], f32,
                              kind="ExternalOutput").ap()

    with tile.TileContext(nc) as tc, ExitStack() as ctx:
        big = ctx.enter_context(tc.tile_pool(name="big", bufs=1))
        cpool = ctx.enter_context(tc.tile_pool(name="cps", bufs=CPBUFS,
                                               space="PSUM"))
        opool = ctx.enter_context(tc.tile_pool(name="ops", bufs=1,
                                               space="PSUM"))

        st_sb = big.tile([128, ST_W], bf16, tag="st")
        # moving data is BAND-MAJOR: per-band contiguous regions (cheap DMA);
        # the matmul rhs reads across bands with a strided AP instead.
        mv_sb = big.tile([128, (2 * T_big + 4 * T_small) * 64], bf16,
                         tag="mv")
        w2_sb = big.tile([128, 1024], bf16, tag="w2")
        c_spread = big.tile([128, B * 512], bf16, tag="csp")
        out_sb = big.tile([128, 2 * U], f32, tag="outsb")
        wu_sb = big.tile([1, 8], bf16, tag="wu")

        # PE warm-up: keep the tensor engine busy through the DMA head so
        # HAM un-throttles (1.2 -> 2.4 GHz) before the real matmuls start
        nc.vector.memset(wu_sb[:], 1.0)
        wups = opool.tile([128, U], f32, tag="po0", name="po0_wu")
        for _ in range(16):
            nc.tensor.matmul(wups[0:1, 0:U], wu_sb[0:1, 0:1],
                             wu_sb[0:1, 0:1].broadcast_to([1, U]),
                             start=True, stop=True)

        bchunks = _chunks(T_big, 3, 4)
        schunks = _chunks(T_small, 2, 2)

        # Engine budget: sync+gpsimd are pure DMA queues; DVE+ACT do only
        # PSUM drains.  ALL moving data is window-major full rectangles with
        # the slot-separation zeros straight from DRAM: every DMA is a
        # contiguous rect, nothing gates anything.
        for b0, b1 in bchunks:
            nc.gpsimd.dma_start(st_sb[:, b0 * 128:b1 * 128],
                                data_dram[:, b0 * 128:b1 * 128])
            nc.sync.dma_start(
                mv_sb[:, b0 * 128:b1 * 128],
                data_dram[:, ST_W + b0 * 128:ST_W + b1 * 128])
        nc.gpsimd.dma_start(
            w2_sb[:], data_dram[:, ST_W + T_big * 128 + T_small * 256:])
        for s0, s1 in schunks:
            nc.gpsimd.dma_start(
                st_sb[:, (T_big + s0) * 128:(T_big + s1) * 128],
                data_dram[:, (T_big + s0) * 128:(T_big + s1) * 128])
            nc.sync.dma_start(
                mv_sb[:, MVB_SB + s0 * 256:MVB_SB + s1 * 256],
                data_dram[:, ST_W + T_big * 128 + s0 * 256:
                          ST_W + T_big * 128 + s1 * 256])

        # C stage; each bank drains as two parallel half-copies (DVE + ACT)
        def drain(ps, b):
            nc.vector.tensor_copy(c_spread[:, b * 512:b * 512 + 256],
                                  ps[:, 0:256])
            nc.scalar.copy(c_spread[:, b * 512 + 256:b * 512 + 512],
                           ps[:, 256:512])

        ps = None
        for t in range(T_big):
            if t % 4 == 0:
                ps = cpool.tile([128, 512], f32, tag="cps")
            nc.tensor.matmul(ps[:, 128 * (t % 4):128 * (t % 4) + 128],
                             st_sb[:, t * 128:(t + 1) * 128],
                             mv_sb[:, t * 128:(t + 1) * 128],
                             start=True, stop=True)
            if t % 4 == 3:
                drain(ps, t // 4)
        for ts in range(T_small):
            if ts % 2 == 0:
                ps = cpool.tile([128, 512], f32, tag="cps")
            nc.tensor.matmul(
                ps[:, 256 * (ts % 2):256 * (ts % 2) + 256],
                st_sb[:, (T_big + ts) * 128:(T_big + ts + 1) * 128],
                mv_sb[:, MVB_SB + ts * 256:MVB_SB + (ts + 1) * 256],
                start=True, stop=True)
            if ts % 2 == 1:
                drain(ps, B_big + ts // 2)

        # W stage: po_g[m, u] += sum_f W[f, m*16+h] * C[u, g, f, h]
        pos = [opool.tile([128, U], f32, tag=f"po{g}", name=f"po{g}")
               for g in range(4)]
        for h in range(HID):
            for g in range(4):       # g inner: 4 strips run concurrently
                nc.tensor.matmul(
                    pos[g][0:MSG, :],
                    w2_sb[32 * g:32 * g + 32, 64 * h:64 * h + 64],
                    c_spread[32 * g:32 * g + 32, (16 * g + h)::64],
                    start=(h == 0), stop=(h == HID - 1),
                    tile_position=(32 * g, 0))
        for g in range(4):
            eng = nc.vector.tensor_copy if g % 2 == 0 else nc.scalar.copy
            eng(out_sb[64 * (g % 2):64 * (g % 2) + MSG,
                       (g // 2) * U:(g // 2 + 1) * U], pos[g][0:MSG, :])
        nc.scalar.dma_start(out_dram[:], out_sb[:])
    nc.compile()
    return nc


def _assemble(outs, segs_per_core, nbig_per_core, T_big, U):
    out = np.zeros((N_NODES, MSG), dtype=np.float32)
    mrow = np.arange(MSG)[None, :]
    for c in range(NCORES):
        segs = segs_per_core[c]
        nbig = nbig_per_core[c]
        P = len(segs)
        if P == 0:
            continue
        po_sb = outs[c].astype(np.float32)           # [128, 2U]
        i = np.arange(P)
        p = np.where(i < nbig, i, 8 * T_big + (i - nbig))  # grid position
        u, g = p // 4, p % 4
        part = 64 * (g % 2)[:, None] + mrow          # [P, 64]
        col = ((g // 2) * U + u)[:, None]
        pos_rows = po_sb[part, col]                  # [P, 64]
        nodes = np.fromiter((segs[k][0] for k in range(P)), dtype=np.int64,
                            count=P)
        np.add.at(out, nodes, pos_rows)
    return out


def kernel(node_features, edge_features, edge_sources, edge_targets,
           hidden, initial, W, b):
    from concourse.bass_utils import run_bass_kernel_spmd

    edge_targets = np.asarray(edge_targets)
    edge_sources = np.asarray(edge_sources)
    edge_features = np.asarray(edge_features, dtype=np.float32)
    hidden = np.asarray(hidden, dtype=np.float32)
    W = np.asarray(W, dtype=np.float32)
    b = np.asarray(b, dtype=np.float32)

    key = edge_targets.tobytes()
    if key in _CACHE:
        layout, nc = _CACHE[key]
    else:
        layout = _build_layout(edge_targets)
        segs_per_core, nbig_per_core, T_big, T_small, U = layout
        nc = _build_program(T_big, T_small, U)
        _CACHE[key] = (layout, nc)
    segs_per_core, nbig_per_core, T_big, T_small, U = layout

    w2 = _build_w2(W)
    in_maps = []
    for c in range(NCORES):
        data = _pack_core(segs_per_core[c], nbig_per_core[c], T_big, T_small,
                          w2, edge_features, edge_sources, hidden)
        in_maps.append({"data": data})

    res = run_bass_kernel_spmd(nc, in_maps, list(range(NCORES)))
    outs = [res.results[c]["out"] for c in range(NCORES)]
    out = _assemble(outs, segs_per_core, nbig_per_core, T_big, U)

    if np.any(b):
        # bias term: out[n] += (sum_{e->n} hidden[src e]) @ Br,
        # Br[h, m] = b[m*16+h].  (b is all-zero for this problem.)
        Br = b.reshape(MSG, HID).T.astype(np.float32)
        acc = np.zeros((N_NODES, HID), dtype=np.float32)
        np.add.at(acc, edge_targets, hidden[edge_sources])
        out += acc @ Br
    return out


# revision 34
# speedup vs baseline: 1.0906x; 1.0906x over previous
"""Trainium2 Bass kernel for nn_MessageLayer (GNN message passing), 8 NeuronCores.

Reference computation:
    edge_mat = (edge_features @ W + b).reshape(E, 64, 16)
    messages = einsum('emh,eh->em', edge_mat, hidden[edge_sources])
    out      = segment_sum(messages, edge_targets, num_segments=10000)

Algebraic restructure (cuts FLOPs 32x): since aggregation is linear,
    out[n, m] = sum_{f,h} W[f, m*16+h] * C[n, f, h],
    C[n, f, h] = sum_{e: tgt(e)=n} ef[e, f] * hidden[src(e), h]

Structure (v2.3): per-target segments ("positions", split at 64) are packed
into full-array K=128 matmuls in two species:
  - BIG (33..64 edges): 2 row-slots of 64 x 4 ef column-classes
    = 8 positions/matmul, moving [128, 128]
  - SMALL (<=32 edges): 4 row-slots of 32 x 4 classes
    = 16 positions/matmul, moving [128, 256]
Stationary [128, 128]: row r of slot j holds the 4 class-edges' features at
column groups 32g..32g+32 (dense).  Moving: slot j's rows carry the 4
source-hidden vectors at cols 64j+16g+h, zeros elsewhere (slot separation;
zeros memset on-device, data DMA'd compactly per slot-band).
PSUM out: valid C-blocks at (32g+f, stripe 16s+h) with s%4 == g uniformly
across both species, garbage elsewhere.  Each bank (4 big or 2 small
matmuls) drains as two half-width [128, 256] f32->bf16 copies (DVE + ACT in
parallel) into the spread c_spread.
W-stage: 4 concurrent 32-row-strip matmul chains (one per class g), each
reading its valid columns via stride-64:  c_spread[32g:32g+32, (16g+h)::64]
-> [32, U], against a 4x-replicated W stationary [32f@32g, 64m] (both
m-halves at once, 16 accumulating h-phases into po_g [64, U] PSUM).

Sharding: node-ownership (scatter-reduce by target): core c owns nodes
[1250c, 1250c+1250) and receives exactly the edges targeting them, so no
collective is needed; host assembles per-position rows into final output.
All tensors bf16 on the wire/SBUF (f32 PSUM accumulate): rel-err ~3.5e-3
vs the 2e-2 gate.
"""
import numpy as np
from contextlib import ExitStack

N_NODES = 10000
N_EDGES = 320000
HID = 16
MSG = 64
EFD = 32
NCORES = 8
NPC = N_NODES // NCORES          # 1250 nodes owned per core
CPBUFS = 4                       # PSUM tiles for C banks (4 + 4 po = 8)

_CACHE = {}


def _bf16():
    import ml_dtypes
    return ml_dtypes.bfloat16


def _build_layout(edge_targets):
    """Per-core position lists (node, edge-ids, len<=64, sorted desc; all
    len>32 "big" positions precede the "small" ones) plus the SPMD-uniform
    grid: T_big 8-position matmuls then T_small 16-position matmuls."""
    segs_per_core, nbig_per_core = [], []
    for c in range(NCORES):
        lo = c * NPC
        mask = (edge_targets >= lo) & (edge_targets < lo + NPC)
        eids = np.nonzero(mask)[0]
        tgt = edge_targets[eids]
        order = np.argsort(tgt, kind="stable")
        eids = eids[order]
        tgt = tgt[order]
        segs = []
        uniq, starts = np.unique(tgt, return_index=True)
        bounds = list(starts) + [len(tgt)]
        for i, n in enumerate(uniq):
            s, e = bounds[i], bounds[i + 1]
            while e - s > 64:
                segs.append((int(n), eids[s:s + 64]))
                s += 64
            segs.append((int(n), eids[s:e]))
        segs.sort(key=lambda t: -len(t[1]))
        segs_per_core.append(segs)
        nbig_per_core.append(sum(1 for _, e in segs if len(e) > 32))

    T_big = -(-max(nbig_per_core) // 8)
    T_big = ((T_big + 3) // 4) * 4            # whole banks of 4 matmuls
    nsmall = max(len(s) - b for s, b in zip(segs_per_core, nbig_per_core))
    T_small = -(-nsmall // 16)
    T_small = ((T_small + 1) // 2) * 2        # whole banks of 2 matmuls
    U = 2 * T_big + 4 * T_small               # total position quads
    assert U <= 512, f"U={U} exceeds one PSUM bank"
    return segs_per_core, nbig_per_core, T_big, T_small, U


def _build_w2(W):
    # w2[32g+f, 64h+m] = W[f, m*16+h], replicated across the 4 class groups
    Wr = W.reshape(EFD, MSG, HID).transpose(0, 2, 1)   # [f, h, m]
    blk = np.ascontiguousarray(Wr.reshape(EFD, HID * MSG))
    return np.tile(blk, (4, 1)).astype(np.float32)     # [128, 1024]


def _pack_core(segs, nbig, T_big, T_small, w2, edge_features, edge_sources,
               hidden):
    """DRAM image per core, bf16:
      [128, T*128 st | T_big*64 mv-big | T_small*64 mv-small | 1024 w2]
    BIG position p<8*T_big (t=p//8, j=(p//4)%2, g=p%4):
      st[64j+r, t*128+32g+f];  mv-big band j at partitions 64j: [64j+r, t*64+16g+h]
    SMALL position q (t=q//16, j=(q//4)%4, g=q%4):
      st[32j+r, (T_big+t)*128+32g+f];  mv-small band j at partitions 32j."""
    T = T_big + T_small
    St = np.zeros((128, T * 128), dtype=np.float32)
    MvB = np.zeros((128, T_big * 64), dtype=np.float32)
    MvS = np.zeros((128, T_small * 64), dtype=np.float32)
    for i in range(len(segs)):
        _, eids = segs[i]
        k = len(eids)
        if i < nbig:
            t, j, g = i // 8, (i // 4) % 2, i % 4
            r0 = 64 * j
            St[r0:r0 + k, t * 128 + 32 * g:t * 128 + 32 * g + EFD] = \
                edge_features[eids]
            MvB[r0:r0 + k, t * 64 + 16 * g:t * 64 + 16 * g + HID] = \
                hidden[edge_sources[eids]]
        else:
            q = i - nbig
            t, j, g = q // 16, (q // 4) % 4, q % 4
            r0 = 32 * j
            St[r0:r0 + k, (T_big + t) * 128 + 32 * g:
               (T_big + t) * 128 + 32 * g + EFD] = edge_features[eids]
            MvS[r0:r0 + k, t * 64 + 16 * g:t * 64 + 16 * g + HID] = \
                hidden[edge_sources[eids]]
    D = np.concatenate([St, MvB, MvS, w2], axis=1)
    return np.ascontiguousarray(D.astype(_bf16()))


def _chunks(T, n, align):
    bs = [((round(k * T / n)) // align) * align for k in range(n)] + [T]
    bs[1] = max(bs[1], align) if T >= align else bs[1]
    return [(bs[k], bs[k + 1]) for k in range(n) if bs[k + 1] > bs[k]]


def _build_program(T_big, T_small, U):
    import concourse.tile as tile
    from concourse import bacc, mybir

    f32 = mybir.dt.float32
    bf16 = mybir.dt.bfloat16
    T = T_big + T_small
    ST_W = T * 128
    B_big = T_big // 4
    B = B_big + T_small // 2             # total PSUM bank-fills
    MVB_SB = T_big * 128                 # big region width in mv_sb

    nc = bacc.Bacc("TRN2", target_bir_lowering=False, debug=False,
                   num_devices=NCORES)
    data_dram = nc.dram_tensor(
        "data", [128, ST_W + (T_big + T_small) * 64 + 1024], bf16,
        kind="ExternalInput").ap()
    out_dram = nc.dram_tensor("out", [128, 2 * U], f32,
                              kind="ExternalOutput").ap()

    with tile.TileContext(nc) as tc, ExitStack() as ctx:
        big = ctx.enter_context(tc.tile_pool(name="big", bufs=1))
        cpool = ctx.enter_context(tc.tile_pool(name="cps", bufs=CPBUFS,
                                               space="PSUM"))
        opool = ctx.enter_context(tc.tile_pool(name="ops", bufs=1,
                                               space="PSUM"))

        st_sb = big.tile([128, ST_W], bf16, tag="st")
        # moving data is BAND-MAJOR: per-band contiguous regions (cheap DMA);
        # the matmul rhs reads across bands with a strided AP instead.
        mv_sb = big.tile([128, (2 * T_big + 4 * T_small) * 64], bf16,
                         tag="mv")
        w2_sb = big.tile([128, 1024], bf16, tag="w2")
        c_spread = big.tile([128, B * 512], bf16, tag="csp")
        out_sb = big.tile([128, 2 * U], f32, tag="outsb")
        wu_sb = big.tile([1, 8], bf16, tag="wu")

        # PE warm-up: keep the tensor engine busy through the DMA head so
        # HAM un-throttles (1.2 -> 2.4 GHz) before the real matmuls start
        nc.vector.memset(wu_sb[:], 1.0)
        wups = opool.tile([128, U], f32, tag="po0", name="po0_wu")
        for _ in range(60):
            nc.tensor.matmul(wups[0:1, 0:1], wu_sb[0:1, 0:1],
                             wu_sb[0:1, 1:2], start=True, stop=True)

        bchunks = _chunks(T_big, 3, 4)
        schunks = _chunks(T_small, 2, 2)

        # moving data is BAND-MAJOR: per-band contiguous regions (cheap DMA);
        # the matmul rhs reads across bands with a strided AP instead.
        # slot-separation zeros (full band regions; the band DMA then
        # overwrites its own rows), split DVE / gpsimd
        for b0, b1 in bchunks:
            nc.vector.memset(mv_sb[:, b0 * 64:b1 * 64], 0.0)
            nc.gpsimd.memset(
                mv_sb[:, MVB_SB // 2 + b0 * 64:MVB_SB // 2 + b1 * 64], 0.0)
        for s0, s1 in schunks:
            for j in range(4):
                off = MVB_SB + (j * T_small + s0) * 64
                eng = nc.vector if j % 2 else nc.gpsimd
                eng.memset(mv_sb[:, off:off + (s1 - s0) * 64], 0.0)

        # stationary + w2 DMAs on the ACT HWDGE queue, moving bands on SP;
        # all band DMAs are contiguous rectangles
        for b0, b1 in bchunks:
            nc.scalar.dma_start(st_sb[:, b0 * 128:b1 * 128],
                                data_dram[:, b0 * 128:b1 * 128])
            for j in range(2):
                off = j * (MVB_SB // 2)
                nc.sync.dma_start(
                    mv_sb[64 * j:64 * j + 64, off + b0 * 64:off + b1 * 64],
                    data_dram[64 * j:64 * j + 64,
                              ST_W + b0 * 64:ST_W + b1 * 64])
        nc.scalar.dma_start(w2_sb[:],
                            data_dram[:, ST_W + (T_big + T_small) * 64:])
        for s0, s1 in schunks:
            nc.scalar.dma_start(
                st_sb[:, (T_big + s0) * 128:(T_big + s1) * 128],
                data_dram[:, (T_big + s0) * 128:(T_big + s1) * 128])
            for j in range(4):
                off = MVB_SB + (j * T_small + s0) * 64
                nc.sync.dma_start(
                    mv_sb[32 * j:32 * j + 32, off:off + (s1 - s0) * 64],
                    data_dram[32 * j:32 * j + 32,
                              ST_W + T_big * 64 + s0 * 64:
                              ST_W + T_big * 64 + s1 * 64])

        # C stage; each bank drains as two parallel half-copies (DVE + ACT)
        def drain(ps, b):
            nc.vector.tensor_copy(c_spread[:, b * 512:b * 512 + 256],
                                  ps[:, 0:256])
            nc.scalar.copy(c_spread[:, b * 512 + 256:b * 512 + 512],
                           ps[:, 256:512])

        mv_big = mv_sb[:, 0:MVB_SB].rearrange("p (r c) -> p r c", r=2)
        mv_small = mv_sb[:, MVB_SB:].rearrange("p (r c) -> p r c", r=4)
        ps = None
        for t in range(T_big):
            if t % 4 == 0:
                ps = cpool.tile([128, 512], f32, tag="cps")
            nc.tensor.matmul(ps[:, 128 * (t % 4):128 * (t % 4) + 128],
                             st_sb[:, t * 128:(t + 1) * 128],
                             mv_big[:, :, t * 64:(t + 1) * 64],
                             start=True, stop=True)
            if t % 4 == 3:
                drain(ps, t // 4)
        for ts in range(T_small):
            if ts % 2 == 0:
                ps = cpool.tile([128, 512], f32, tag="cps")
            nc.tensor.matmul(
                ps[:, 256 * (ts % 2):256 * (ts % 2) + 256],
                st_sb[:, (T_big + ts) * 128:(T_big + ts + 1) * 128],
                mv_small[:, :, ts * 64:(ts + 1) * 64],
                start=True, stop=True)
            if ts % 2 == 1:
                drain(ps, B_big + ts // 2)

        # W stage: po_g[m, u] += sum_f W[f, m*16+h] * C[u, g, f, h]
        pos = [opool.tile([128, U], f32, tag=f"po{g}", name=f"po{g}")
               for g in range(4)]
        for h in range(HID):
            for g in range(4):       # g inner: 4 strips run concurrently
                nc.tensor.matmul(
                    pos[g][0:MSG, :],
                    w2_sb[32 * g:32 * g + 32, 64 * h:64 * h + 64],
                    c_spread[32 * g:32 * g + 32, (16 * g + h)::64],
                    start=(h == 0), stop=(h == HID - 1),
                    tile_position=(32 * g, 0))
        for g in range(4):
            eng = nc.vector.tensor_copy if g % 2 == 0 else nc.scalar.copy
            eng(out_sb[64 * (g % 2):64 * (g % 2) + MSG,
                       (g // 2) * U:(g // 2 + 1) * U], pos[g][0:MSG, :])
        nc.sync.dma_start(out_dram[:], out_sb[:])
    nc.compile()
    return nc


def _assemble(outs, segs_per_core, nbig_per_core, T_big, U):
    out = np.zeros((N_NODES, MSG), dtype=np.float32)
    mrow = np.arange(MSG)[None, :]
    for c in range(NCORES):
        segs = segs_per_core[c]
        nbig = nbig_per_core[c]
        P = len(segs)
        if P == 0:
            continue
        po_sb = outs[c].astype(np.float32)           # [128, 2U]
        i = np.arange(P)
        p = np.where(i < nbig, i, 8 * T_big + (i - nbig))  # grid position
        u, g = p // 4, p % 4
        part = 64 * (g % 2)[:, None] + mrow          # [P, 64]
        col = ((g // 2) * U + u)[:, None]
        pos_rows = po_sb[part, col]                  # [P, 64]
        nodes = np.fromiter((segs[k][0] for k in range(P)), dtype=np.int64,
                            count=P)
        np.add.at(out, nodes, pos_rows)
    return out


def kernel(node_features, edge_features, edge_sources, edge_targets,
           hidden, initial, W, b):
    from concourse.bass_utils import run_bass_kernel_spmd

    edge_targets = np.asarray(edge_targets)
    edge_sources = np.asarray(edge_sources)
    edge_features = np.asarray(edge_features, dtype=np.float32)
    hidden = np.asarray(hidden, dtype=np.float32)
    W = np.asarray(W, dtype=np.float32)
    b = np.asarray(b, dtype=np.float32)

    key = edge_targets.tobytes()
    if key in _CACHE:
        layout, nc = _CACHE[key]
    else:
        layout = _build_layout(edge_targets)
        segs_per_core, nbig_per_core, T_big, T_small, U = layout
        nc = _build_program(T_big, T_small, U)
        _CACHE[key] = (layout, nc)
    segs_per_core, nbig_per_core, T_big, T_small, U = layout

    w2 = _build_w2(W)
    in_maps = []
    for c in range(NCORES):
        data = _pack_core(segs_per_core[c], nbig_per_core[c], T_big, T_small,
                          w2, edge_features, edge_sources, hidden)
        in_maps.append({"data": data})

    res = run_bass_kernel_spmd(nc, in_maps, list(range(NCORES)))
    outs = [res.results[c]["out"] for c in range(NCORES)]
    out = _assemble(outs, segs_per_core, nbig_per_core, T_big, U)

    if np.any(b):
        # bias term: out[n] += (sum_{e->n} hidden[src e]) @ Br,
        # Br[h, m] = b[m*16+h].  (b is all-zero for this problem.)
        Br = b.reshape(MSG, HID).T.astype(np.float32)
        acc = np.zeros((N_NODES, HID), dtype=np.float32)
        np.add.at(acc, edge_targets, hidden[edge_sources])
        out += acc @ Br
    return out


# revision 35
# speedup vs baseline: 1.1755x; 1.0779x over previous
"""Trainium2 Bass kernel for nn_MessageLayer (GNN message passing), 8 NeuronCores.

Reference computation:
    edge_mat = (edge_features @ W + b).reshape(E, 64, 16)
    messages = einsum('emh,eh->em', edge_mat, hidden[edge_sources])
    out      = segment_sum(messages, edge_targets, num_segments=10000)

Algebraic restructure (cuts FLOPs 32x): since aggregation is linear,
    out[n, m] = sum_{f,h} W[f, m*16+h] * C[n, f, h],
    C[n, f, h] = sum_{e: tgt(e)=n} ef[e, f] * hidden[src(e), h]

Structure (v2.3): per-target segments ("positions", split at 64) are packed
into full-array K=128 matmuls in two species:
  - BIG (33..64 edges): 2 row-slots of 64 x 4 ef column-classes
    = 8 positions/matmul, moving [128, 128]
  - SMALL (<=32 edges): 4 row-slots of 32 x 4 classes
    = 16 positions/matmul, moving [128, 256]
Stationary [128, 128]: row r of slot j holds the 4 class-edges' features at
column groups 32g..32g+32 (dense).  Moving: slot j's rows carry the 4
source-hidden vectors at cols 64j+16g+h, zeros elsewhere (slot separation;
zeros memset on-device, data DMA'd compactly per slot-band).
PSUM out: valid C-blocks at (32g+f, stripe 16s+h) with s%4 == g uniformly
across both species, garbage elsewhere.  Each bank (4 big or 2 small
matmuls) drains as two half-width [128, 256] f32->bf16 copies (DVE + ACT in
parallel) into the spread c_spread.
W-stage: 4 concurrent 32-row-strip matmul chains (one per class g), each
reading its valid columns via stride-64:  c_spread[32g:32g+32, (16g+h)::64]
-> [32, U], against a 4x-replicated W stationary [32f@32g, 64m] (both
m-halves at once, 16 accumulating h-phases into po_g [64, U] PSUM).

Sharding: node-ownership (scatter-reduce by target): core c owns nodes
[1250c, 1250c+1250) and receives exactly the edges targeting them, so no
collective is needed; host assembles per-position rows into final output.
All tensors bf16 on the wire/SBUF (f32 PSUM accumulate): rel-err ~3.5e-3
vs the 2e-2 gate.
"""
import numpy as np
from contextlib import ExitStack

N_NODES = 10000
N_EDGES = 320000
HID = 16
MSG = 64
EFD = 32
NCORES = 8
NPC = N_NODES // NCORES          # 1250 nodes owned per core
CPBUFS = 4                       # PSUM tiles for C banks (4 + 4 po = 8)

_CACHE = {}


def _bf16():
    import ml_dtypes
    return ml_dtypes.bfloat16


def _build_layout(edge_targets):
    """Per-core position lists (node, edge-ids, len<=64, sorted desc; all
    len>32 "big" positions precede the "small" ones) plus the SPMD-uniform
    grid: T_big 8-position matmuls then T_small 16-position matmuls."""
    segs_per_core, nbig_per_core = [], []
    for c in range(NCORES):
        lo = c * NPC
        mask = (edge_targets >= lo) & (edge_targets < lo + NPC)
        eids = np.nonzero(mask)[0]
        tgt = edge_targets[eids]
        order = np.argsort(tgt, kind="stable")
        eids = eids[order]
        tgt = tgt[order]
        segs = []
        uniq, starts = np.unique(tgt, return_index=True)
        bounds = list(starts) + [len(tgt)]
        for i, n in enumerate(uniq):
            s, e = bounds[i], bounds[i + 1]
            while e - s > 64:
                segs.append((int(n), eids[s:s + 64]))
                s += 64
            segs.append((int(n), eids[s:e]))
        segs.sort(key=lambda t: -len(t[1]))
        segs_per_core.append(segs)
        nbig_per_core.append(sum(1 for _, e in segs if len(e) > 32))

    T_big = -(-max(nbig_per_core) // 8)
    T_big = ((T_big + 3) // 4) * 4            # whole banks of 4 matmuls
    nsmall = max(len(s) - b for s, b in zip(segs_per_core, nbig_per_core))
    T_small = -(-nsmall // 16)
    T_small = ((T_small + 1) // 2) * 2        # whole banks of 2 matmuls
    U = 2 * T_big + 4 * T_small               # total position quads
    assert U <= 512, f"U={U} exceeds one PSUM bank"
    return segs_per_core, nbig_per_core, T_big, T_small, U


def _build_w2(W):
    # w2[32g+f, 64h+m] = W[f, m*16+h], replicated across the 4 class groups
    Wr = W.reshape(EFD, MSG, HID).transpose(0, 2, 1)   # [f, h, m]
    blk = np.ascontiguousarray(Wr.reshape(EFD, HID * MSG))
    return np.tile(blk, (4, 1)).astype(np.float32)     # [128, 1024]


def _pack_core(segs, nbig, T_big, T_small, w2, edge_features, edge_sources,
               hidden):
    """DRAM image per core, bf16:
      [128, T*128 st | T_big*64 mv-big | T_small*64 mv-small | 1024 w2]
    BIG position p<8*T_big (t=p//8, j=(p//4)%2, g=p%4):
      st[64j+r, t*128+32g+f];  mv-big band j at partitions 64j: [64j+r, t*64+16g+h]
    SMALL position q (t=q//16, j=(q//4)%4, g=q%4):
      st[32j+r, (T_big+t)*128+32g+f];  mv-small band j at partitions 32j."""
    T = T_big + T_small
    St = np.zeros((128, T * 128), dtype=np.float32)
    MvB = np.zeros((128, T_big * 64), dtype=np.float32)
    MvS = np.zeros((128, T_small * 64), dtype=np.float32)
    for i in range(len(segs)):
        _, eids = segs[i]
        k = len(eids)
        if i < nbig:
            t, j, g = i // 8, (i // 4) % 2, i % 4
            r0 = 64 * j
            St[r0:r0 + k, t * 128 + 32 * g:t * 128 + 32 * g + EFD] = \
                edge_features[eids]
            MvB[r0:r0 + k, t * 64 + 16 * g:t * 64 + 16 * g + HID] = \
                hidden[edge_sources[eids]]
        else:
            q = i - nbig
            t, j, g = q // 16, (q // 4) % 4, q % 4
            r0 = 32 * j
            St[r0:r0 + k, (T_big + t) * 128 + 32 * g:
               (T_big + t) * 128 + 32 * g + EFD] = edge_features[eids]
            MvS[r0:r0 + k, t * 64 + 16 * g:t * 64 + 16 * g + HID] = \
                hidden[edge_sources[eids]]
    D = np.concatenate([St, MvB, MvS, w2], axis=1)
    return np.ascontiguousarray(D.astype(_bf16()))


def _chunks(T, n, align):
    bs = [((round(k * T / n)) // align) * align for k in range(n)] + [T]
    bs[1] = max(bs[1], align) if T >= align else bs[1]
    return [(bs[k], bs[k + 1]) for k in range(n) if bs[k + 1] > bs[k]]


def _build_program(T_big, T_small, U):
    import concourse.tile as tile
    from concourse import bacc, mybir

    f32 = mybir.dt.float32
    bf16 = mybir.dt.bfloat16
    T = T_big + T_small
    ST_W = T * 128
    B_big = T_big // 4
    B = B_big + T_small // 2             # total PSUM bank-fills
    MVB_SB = T_big * 128                 # big region width in mv_sb

    nc = bacc.Bacc("TRN2", target_bir_lowering=False, debug=False,
                   num_devices=NCORES)
    data_dram = nc.dram_tensor(
        "data", [128, ST_W + (T_big + T_small) * 64 + 1024], bf16,
        kind="ExternalInput").ap()
    out_dram = nc.dram_tensor("out", [128, 2 * U], f32,
                              kind="ExternalOutput").ap()

    with tile.TileContext(nc) as tc, ExitStack() as ctx:
        big = ctx.enter_context(tc.tile_pool(name="big", bufs=1))
        cpool = ctx.enter_context(tc.tile_pool(name="cps", bufs=CPBUFS,
                                               space="PSUM"))
        opool = ctx.enter_context(tc.tile_pool(name="ops", bufs=1,
                                               space="PSUM"))

        st_sb = big.tile([128, ST_W], bf16, tag="st")
        # moving data is BAND-MAJOR: per-band contiguous regions (cheap DMA);
        # the matmul rhs reads across bands with a strided AP instead.
        mv_sb = big.tile([128, (2 * T_big + 4 * T_small) * 64], bf16,
                         tag="mv")
        w2_sb = big.tile([128, 1024], bf16, tag="w2")
        c_spread = big.tile([128, B * 512], bf16, tag="csp")
        out_sb = big.tile([128, 2 * U], f32, tag="outsb")
        wu_sb = big.tile([1, 8], bf16, tag="wu")

        # PE warm-up: keep the tensor engine busy through the DMA head so
        # HAM un-throttles (1.2 -> 2.4 GHz) before the real matmuls start
        nc.vector.memset(wu_sb[:], 1.0)
        wups = opool.tile([128, U], f32, tag="po0", name="po0_wu")
        for _ in range(38):
            nc.tensor.matmul(wups[0:1, 0:U], wu_sb[0:1, 0:1],
                             wu_sb[0:1, 0:1].broadcast_to([1, U]),
                             start=True, stop=True)

        bchunks = _chunks(T_big, 3, 4)
        schunks = _chunks(T_small, 2, 2)

        # moving data is BAND-MAJOR: per-band contiguous regions (cheap DMA);
        # the matmul rhs reads across bands with a strided AP instead.
        # slot-separation zeros (full band regions; the band DMA then
        # overwrites its own rows), split DVE / gpsimd
        for b0, b1 in bchunks:
            nc.vector.memset(mv_sb[:, b0 * 64:b1 * 64], 0.0)
            nc.gpsimd.memset(
                mv_sb[:, MVB_SB // 2 + b0 * 64:MVB_SB // 2 + b1 * 64], 0.0)
        for s0, s1 in schunks:
            for j in range(4):
                off = MVB_SB + (j * T_small + s0) * 64
                eng = nc.vector if j % 2 else nc.gpsimd
                eng.memset(mv_sb[:, off:off + (s1 - s0) * 64], 0.0)

        # stationary + w2 DMAs on the ACT HWDGE queue, moving bands on SP;
        # all band DMAs are contiguous rectangles
        for b0, b1 in bchunks:
            nc.scalar.dma_start(st_sb[:, b0 * 128:b1 * 128],
                                data_dram[:, b0 * 128:b1 * 128])
            for j in range(2):
                off = j * (MVB_SB // 2)
                nc.sync.dma_start(
                    mv_sb[64 * j:64 * j + 64, off + b0 * 64:off + b1 * 64],
                    data_dram[64 * j:64 * j + 64,
                              ST_W + b0 * 64:ST_W + b1 * 64])
        nc.scalar.dma_start(w2_sb[:],
                            data_dram[:, ST_W + (T_big + T_small) * 64:])
        for s0, s1 in schunks:
            nc.scalar.dma_start(
                st_sb[:, (T_big + s0) * 128:(T_big + s1) * 128],
                data_dram[:, (T_big + s0) * 128:(T_big + s1) * 128])
            for j in range(4):
                off = MVB_SB + (j * T_small + s0) * 64
                nc.sync.dma_start(
                    mv_sb[32 * j:32 * j + 32, off:off + (s1 - s0) * 64],
                    data_dram[32 * j:32 * j + 32,
                              ST_W + T_big * 64 + s0 * 64:
                              ST_W + T_big * 64 + s1 * 64])

        # C stage; each bank drains as two parallel half-copies (DVE + ACT)
        def drain(ps, b):
            nc.vector.tensor_copy(c_spread[:, b * 512:b * 512 + 256],
                                  ps[:, 0:256])
            nc.scalar.copy(c_spread[:, b * 512 + 256:b * 512 + 512],
                           ps[:, 256:512])

        mv_big = mv_sb[:, 0:MVB_SB].rearrange("p (r c) -> p r c", r=2)
        mv_small = mv_sb[:, MVB_SB:].rearrange("p (r c) -> p r c", r=4)
        ps = None
        for t in range(T_big):
            if t % 4 == 0:
                ps = cpool.tile([128, 512], f32, tag="cps")
            nc.tensor.matmul(ps[:, 128 * (t % 4):128 * (t % 4) + 128],
                             st_sb[:, t * 128:(t + 1) * 128],
                             mv_big[:, :, t * 64:(t + 1) * 64],
                             start=True, stop=True)
            if t % 4 == 3:
                drain(ps, t // 4)
        for ts in range(T_small):
            if ts % 2 == 0:
                ps = cpool.tile([128, 512], f32, tag="cps")
            nc.tensor.matmul(
                ps[:, 256 * (ts % 2):256 * (ts % 2) + 256],
                st_sb[:, (T_big + ts) * 128:(T_big + ts + 1) * 128],
                mv_small[:, :, ts * 64:(ts + 1) * 64],
                start=True, stop=True)
            if ts % 2 == 1:
                drain(ps, B_big + ts // 2)

        # W stage: po_g[m, u] += sum_f W[f, m*16+h] * C[u, g, f, h]
        pos = [opool.tile([128, U], f32, tag=f"po{g}", name=f"po{g}")
               for g in range(4)]
        for h in range(HID):
            for g in range(4):       # g inner: 4 strips run concurrently
                nc.tensor.matmul(
                    pos[g][0:MSG, :],
                    w2_sb[32 * g:32 * g + 32, 64 * h:64 * h + 64],
                    c_spread[32 * g:32 * g + 32, (16 * g + h)::64],
                    start=(h == 0), stop=(h == HID - 1),
                    tile_position=(32 * g, 0))
        for g in range(4):
            eng = nc.vector.tensor_copy if g % 2 == 0 else nc.scalar.copy
            eng(out_sb[64 * (g % 2):64 * (g % 2) + MSG,
                       (g // 2) * U:(g // 2 + 1) * U], pos[g][0:MSG, :])
        nc.sync.dma_start(out_dram[:], out_sb[:])
    nc.compile()
    return nc


def _assemble(outs, segs_per_core, nbig_per_core, T_big, U):
    out = np.zeros((N_NODES, MSG), dtype=np.float32)
    mrow = np.arange(MSG)[None, :]
    for c in range(NCORES):
        segs = segs_per_core[c]
        nbig = nbig_per_core[c]
        P = len(segs)
        if P == 0:
            continue
        po_sb = outs[c].astype(np.float32)           # [128, 2U]
        i = np.arange(P)
        p = np.where(i < nbig, i, 8 * T_big + (i - nbig))  # grid position
        u, g = p // 4, p % 4
        part = 64 * (g % 2)[:, None] + mrow          # [P, 64]
        col = ((g // 2) * U + u)[:, None]
        pos_rows = po_sb[part, col]                  # [P, 64]
        nodes = np.fromiter((segs[k][0] for k in range(P)), dtype=np.int64,
                            count=P)
        np.add.at(out, nodes, pos_rows)
    return out


def kernel(node_features, edge_features, edge_sources, edge_targets,
           hidden, initial, W, b):
    from concourse.bass_utils import run_bass_kernel_spmd

    edge_targets = np.asarray(edge_targets)
    edge_sources = np.asarray(edge_sources)
    edge_features = np.asarray(edge_features, dtype=np.float32)
    hidden = np.asarray(hidden, dtype=np.float32)
    W = np.asarray(W, dtype=np.float32)
    b = np.asarray(b, dtype=np.float32)

    key = edge_targets.tobytes()
    if key in _CACHE:
        layout, nc = _CACHE[key]
    else:
        layout = _build_layout(edge_targets)
        segs_per_core, nbig_per_core, T_big, T_small, U = layout
        nc = _build_program(T_big, T_small, U)
        _CACHE[key] = (layout, nc)
    segs_per_core, nbig_per_core, T_big, T_small, U = layout

    w2 = _build_w2(W)
    in_maps = []
    for c in range(NCORES):
        data = _pack_core(segs_per_core[c], nbig_per_core[c], T_big, T_small,
                          w2, edge_features, edge_sources, hidden)
        in_maps.append({"data": data})

    res = run_bass_kernel_spmd(nc, in_maps, list(range(NCORES)))
    outs = [res.results[c]["out"] for c in range(NCORES)]
    out = _assemble(outs, segs_per_core, nbig_per_core, T_big, U)

    if np.any(b):
        # bias term: out[n] += (sum_{e->n} hidden[src e]) @ Br,
        # Br[h, m] = b[m*16+h].  (b is all-zero for this problem.)
        Br = b.reshape(MSG, HID).T.astype(np.float32)
        acc = np.zeros((N_NODES, HID), dtype=np.float32)
        np.add.at(acc, edge_targets, hidden[edge_sources])
        out += acc @ Br
    return out
